# revision 12
# baseline (speedup 1.0000x reference)
"""Single-head attention (B=8, S=4096, E=512, H=64) on 8 trn2 NeuronCores.

Sharding: data-parallel over batch — one batch element per core.

Per-core algorithm (batch b):
  - Host pre-transposes x[b] -> xT [E, S] (f32r bits) and converts the
    int32 mask to bf16 {0,1} (exact).  Mask HBM traffic is 32 MB/core
    instead of 64, and the device needs no int32->bf16 cast pass (the
    baseline burned ~97us of DVE on those casts).
  - QKV: Q^T,K^T [H, S] head-major and V' [S, H+1] S-major (ones column
    appended), all f32r, via PE matmuls over E-chunks; weights/x DMA
    straight into f32r tiles (no cast).
  - Scores computed TRANSPOSED: S^T[sk, sq] = K^T.T @ Q^T so softmax runs
    along partitions and attn @ V needs no transpose of attn.
  - Mask applied additively PRE-exp using the PE's free lhsT transpose:
    S^T += mask_chunk.T @ (-32768 * I).  All elementwise two-tensor ops
    (DVE tensor_tensor / scalar_tensor_tensor, Pool tensor_tensor) are
    avoided on purpose: on this hardware they pair-accumulate
    (out[2i] = a[2i]b[2i] + a[2i+1]b[2i+1]) on lanes 84-95/116-127 under
    concurrent load, on every dtype combination tested.
  - exp on ACT with no max-subtraction (|scaled scores| < ~10, safe),
    f32r out; exp(scale*(qk - 32768*m)) underflows to exactly 0 on
    masked lanes.
  - Softmax denominator comes free from the ones column of V':
    outT = V'.T @ attn^T accumulates [H+1, sq] where row H is the row sum.
  - Fixup per q block: 4 batched PE transposes into ONE PSUM bank tile,
    reciprocal + scale on DVE (per-partition-scalar ops only), one
    gathered DMA out.

Phase B runs as one flat pipeline over all 128 (qb, g) groups with attn@V
trailing the scores/exp chain by TRAIL groups, so the PE (the bottleneck
engine) never stalls and stays at the 2.4 GHz pstate.
"""
import sys

sys.path.insert(0, "/opt/trn_rl_repo")

import ml_dtypes
import numpy as np

import concourse.bacc as bacc
import concourse.tile as tile
from concourse import mybir
from concourse.bass_utils import run_bass_kernel_spmd

F32 = mybir.dt.float32
F32R = mybir.dt.float32r
BF16 = mybir.dt.bfloat16

B, S, E, H = 8, 4096, 512, 64
SCALE = float(E) ** -0.5
NEG = -32768.0

BF16NP = ml_dtypes.bfloat16

TRAIL = 2   # attn@V trails scores/exp by this many [128,1024] groups


def build_program(s=S):
    nc = bacc.Bacc("TRN2", target_bir_lowering=False, debug=False, num_devices=B)
    xT = nc.dram_tensor("xT", [E, s], F32R, kind="ExternalInput")
    maskb = nc.dram_tensor("maskb", [s, s], BF16, kind="ExternalInput")
    wq = nc.dram_tensor("wq", [E, H], F32R, kind="ExternalInput")
    wk = nc.dram_tensor("wk", [E, H], F32R, kind="ExternalInput")
    wv = nc.dram_tensor("wv", [E, H], F32R, kind="ExternalInput")
    bqt = nc.dram_tensor("bqt", [H, 1], F32, kind="ExternalInput")
    bkt = nc.dram_tensor("bkt", [H, 1], F32, kind="ExternalInput")
    bv1 = nc.dram_tensor("bv1", [1, H + 1], F32, kind="ExternalInput")
    out = nc.dram_tensor("out", [s, H], F32, kind="ExternalOutput")

    NE = E // 128          # 4 E-chunks
    NB = s // 512          # q/s blocks of 512
    NQ = s // 128          # 128-row chunks
    NG = NQ // 2           # [128,1024]-score groups per q block
    GQ = NB * NG           # total groups

    with tile.TileContext(nc) as tc:
        with (
            tc.tile_pool(name="const", bufs=1) as cst,
            tc.tile_pool(name="xp", bufs=2) as xp,
            tc.tile_pool(name="qkv", bufs=1) as qkv,
            tc.tile_pool(name="maskp", bufs=7) as maskp,
            tc.tile_pool(name="etp", bufs=3) as etp,
            tc.tile_pool(name="osb", bufs=2) as osb,
        ):
            # ---- constants ----
            negI = cst.tile([128, 128], BF16)
            nc.gpsimd.memset(negI, 0.0)
            nc.gpsimd.affine_select(
                out=negI, in_=negI, compare_op=mybir.AluOpType.not_equal,
                fill=NEG, base=0, pattern=[[-1, 128]], channel_multiplier=1,
            )
            idf = cst.tile([128, 128], F32)
            nc.gpsimd.memset(idf, 0.0)
            nc.gpsimd.affine_select(
                out=idf, in_=idf, compare_op=mybir.AluOpType.not_equal,
                fill=1.0, base=0, pattern=[[-1, 128]], channel_multiplier=1,
            )
            ones128 = cst.tile([1, 128], F32)
            nc.vector.memset(ones128, 1.0)

            wq_r = cst.tile([128, NE, H], F32R)
            wk_r = cst.tile([128, NE, H], F32R)
            wv_r = cst.tile([128, NE, H], F32R)
            for w_dram, w_r in ((wq, wq_r), (wk, wk_r), (wv, wv_r)):
                nc.sync.dma_start(
                    out=w_r, in_=w_dram.rearrange("(c p) h -> p c h", p=128)
                )
            bv1_sb = cst.tile([1, H + 1], F32)
            nc.sync.dma_start(out=bv1_sb, in_=bv1[:])
            bqt_sb = cst.tile([H, 1], F32)
            bkt_sb = cst.tile([H, 1], F32)
            nc.sync.dma_start(out=bqt_sb, in_=bqt[:])
            nc.sync.dma_start(out=bkt_sb, in_=bkt[:])

            # ---- mask DMA (issue first: no deps, fills DMA queues early) ----
            # mbs[qb][j]: mask rows for sq chunk (qb, j) as bf16 {0,1},
            # [128, s] — plain 2D row-slice DMAs.
            mbs = []
            for qb in range(NB):
                row = []
                for j in range(4):
                    q0 = qb * 512 + j * 128
                    mb = maskp.tile([128, s], BF16, tag="mb", name=f"mb_{qb}_{j}")
                    nc.sync.dma_start(out=mb, in_=maskb[q0:q0 + 128, :])
                    row.append(mb)
                mbs.append(row)

            # ---- phase A: QT, KT head-major; V' S-major (all f32r) ----
            QTb = [qkv.tile([H, 512], F32R, name=f"qt_{i}") for i in range(NB)]
            KTb = [qkv.tile([H, 512], F32R, name=f"kt_{i}") for i in range(NB)]
            VPk = [qkv.tile([128, H + 1], F32R, name=f"vp_{i}") for i in range(NQ)]
            with tc.tile_pool(name="psA", bufs=2, space="PSUM") as psA:
                for sb in range(NB):
                    s0 = sb * 512
                    xtr = xp.tile([128, NE, 512], F32R, tag="xtr", name=f"xtr_{sb}")
                    half = NE // 2
                    for eh in range(2):
                        e0 = eh * half
                        eng = nc.scalar if eh == 0 else nc.gpsimd
                        eng.dma_start(
                            out=xtr[:, e0:e0 + half, :],
                            in_=xT[e0 * 128:(e0 + half) * 128, s0:s0 + 512]
                            .rearrange("(c p) s -> p c s", p=128),
                        )
                    q_ps = psA.tile([H, 512], F32, tag="qk", name=f"q_ps_{sb}")
                    k_ps = psA.tile([H, 512], F32, tag="qk", name=f"k_ps_{sb}")
                    for e in range(NE):
                        nc.tensor.matmul(q_ps, wq_r[:, e, :], xtr[:, e, :],
                                         start=(e == 0), stop=(e == NE - 1))
                        nc.tensor.matmul(k_ps, wk_r[:, e, :], xtr[:, e, :],
                                         start=(e == 0), stop=(e == NE - 1))
                    nc.scalar.activation(QTb[sb], q_ps,
                                         mybir.ActivationFunctionType.Identity,
                                         bias=bqt_sb)
                    nc.scalar.activation(KTb[sb], k_ps,
                                         mybir.ActivationFunctionType.Identity,
                                         bias=bkt_sb)
                    for j0 in range(0, 4, 2):
                        vps = [
                            psA.tile([128, H + 1], F32, tag="v",
                                     name=f"v_ps_{sb}_{j0 + jj}")
                            for jj in range(2)
                        ]
                        for jj in range(2):
                            nc.tensor.matmul(vps[jj], ones128, bv1_sb,
                                             start=True, stop=False)
                        for e in range(NE):
                            for jj in range(2):
                                c0 = (j0 + jj) * 128
                                nc.tensor.matmul(
                                    vps[jj][:, 0:H], xtr[:, e, c0:c0 + 128],
                                    wv_r[:, e, :], start=False, stop=(e == NE - 1),
                                )
                        for jj in range(2):
                            nc.vector.tensor_copy(VPk[sb * 4 + j0 + jj], vps[jj])

            # ---- phase B: flat pipeline over all (qb, g) groups ----
            with (
                tc.tile_pool(name="psS", bufs=2, space="PSUM") as psS,
                tc.tile_pool(name="psO", bufs=2, space="PSUM") as psO,
                tc.tile_pool(name="psF", bufs=2, space="PSUM") as psF,
            ):
                ot_ps = [None] * NB

                def scores(G):
                    qb, g = divmod(G, NG)
                    sc = psS.tile([128, 1024], F32, tag="sc", name=f"sc_{G}")
                    for h2 in range(2):
                        k = 2 * g + h2
                        nc.tensor.matmul(
                            sc[:, 512 * h2:512 * h2 + 512],
                            KTb[k // 4][:, 128 * (k % 4):128 * (k % 4 + 1)],
                            QTb[qb],
                            start=True, stop=False,
                        )
                    for j in range(4):
                        for h2 in range(2):
                            k = 2 * g + h2
                            c = 512 * h2 + 128 * j
                            nc.tensor.matmul(
                                sc[:, c:c + 128],
                                mbs[qb][j][:, 128 * k:128 * (k + 1)],
                                negI, start=False, stop=(j == 3),
                            )
                    return sc

                def expg(G, sc):
                    et = etp.tile([128, 1024], F32R, tag="et")
                    nc.scalar.activation(
                        et, sc, mybir.ActivationFunctionType.Exp, scale=SCALE
                    )
                    return et

                def attnv(G, et):
                    qb, g = divmod(G, NG)
                    if ot_ps[qb] is None:
                        ot_ps[qb] = psO.tile([H + 1, 512], F32, tag="ot",
                                             name=f"ot_{qb}")
                    for h2 in range(2):
                        k = 2 * g + h2
                        nc.tensor.matmul(
                            ot_ps[qb], VPk[k], et[:, 512 * h2:512 * h2 + 512],
                            start=(k == 0), stop=(k == NQ - 1),
                        )

                def fixup(qb):
                    q0 = qb * 512
                    oT = osb.tile([H + 1, 512], F32, tag="oT")
                    nc.vector.tensor_copy(oT, ot_ps[qb])
                    fx = psF.tile([128, 4, H + 1], F32, tag="fx")
                    for j in range(4):
                        nc.tensor.transpose(
                            fx[:, j, :], oT[:, 128 * j:128 * (j + 1)],
                            idf[0:H + 1, 0:H + 1]
                        )
                    ob = osb.tile([128, 4, H + 1], F32, tag="ob")
                    nc.vector.tensor_copy(ob, fx)
                    rc = osb.tile([128, 4], F32, tag="rc")
                    nc.vector.reciprocal(rc, ob[:, :, H])
                    of = osb.tile([128, 4, H], F32, tag="of")
                    for j in range(4):
                        nc.vector.tensor_scalar_mul(
                            of[:, j, :], ob[:, j, 0:H], rc[:, j:j + 1]
                        )
                    nc.gpsimd.dma_start(
                        out=out[q0:q0 + 512, :].rearrange("(j p) h -> p j h", p=128),
                        in_=of,
                    )

                ets = {}
                scn = {0: scores(0)}
                for G in range(GQ):
                    if G + 1 < GQ:
                        scn[G + 1] = scores(G + 1)
                    ets[G] = expg(G, scn.pop(G))
                    if G - TRAIL >= 0:
                        attnv(G - TRAIL, ets.pop(G - TRAIL))
                        if (G - TRAIL) % NG == NG - 1:
                            fixup((G - TRAIL) // NG)
                for G in range(GQ - TRAIL, GQ):
                    attnv(G, ets.pop(G))
                    if G % NG == NG - 1:
                        fixup(G // NG)
    nc.compile()
    return nc


def make_in_maps(x, attention_mask, Wq, bq, Wk, bk, Wv, bv):
    nb = x.shape[0]
    bv1 = np.concatenate([bv, np.ones(1, np.float32)]).reshape(1, H + 1)
    common = {
        "wq": np.ascontiguousarray(Wq), "wk": np.ascontiguousarray(Wk),
        "wv": np.ascontiguousarray(Wv),
        "bqt": np.ascontiguousarray(bq.reshape(H, 1)),
        "bkt": np.ascontiguousarray(bk.reshape(H, 1)),
        "bv1": bv1,
    }
    return [
        {
            "xT": np.ascontiguousarray(x[b].T),
            # mask as bf16 {0,1}, exact
            "maskb": (attention_mask[b] != 0).astype(BF16NP),
            **common,
        }
        for b in range(nb)
    ]


_PROGRAM = None


def kernel(x, attention_mask, Wq, bq, Wk, bk, Wv, bv):
    global _PROGRAM
    x = np.asarray(x, np.float32)
    attention_mask = np.asarray(attention_mask, np.int32)
    if _PROGRAM is None:
        _PROGRAM = build_program()
    in_maps = make_in_maps(
        x, attention_mask,
        np.asarray(Wq, np.float32), np.asarray(bq, np.float32),
        np.asarray(Wk, np.float32), np.asarray(bk, np.float32),
        np.asarray(Wv, np.float32), np.asarray(bv, np.float32),
    )
    res = run_bass_kernel_spmd(_PROGRAM, in_maps, core_ids=list(range(B)))
    return np.stack([res.results[b]["out"] for b in range(B)], axis=0)


# revision 13
# speedup vs baseline: 1.1327x; 1.1327x over previous
"""Single-head attention (B=8, S=4096, E=512, H=64) on 8 trn2 NeuronCores.

Sharding: data-parallel over batch — one batch element per core.

Per-core algorithm (batch b):
  - Host pre-transposes x[b] -> xT [E, S] (f32r bits) and converts the
    int32 mask to bf16 {0,1} (exact).  Mask HBM traffic is 32 MB/core
    instead of 64, and the device needs no int32->bf16 cast pass (the
    baseline burned ~97us of DVE on those casts).
  - QKV: Q^T,K^T [H, S] head-major and V' [S, H+1] S-major (ones column
    appended), all f32r, via PE matmuls over E-chunks; weights/x DMA
    straight into f32r tiles (no cast).
  - Scores computed TRANSPOSED: S^T[sk, sq] = K^T.T @ Q^T so softmax runs
    along partitions and attn @ V needs no transpose of attn.
  - Mask applied additively PRE-exp using the PE's free lhsT transpose:
    S^T += mask_chunk.T @ (-32768 * I).  All elementwise two-tensor ops
    (DVE tensor_tensor / scalar_tensor_tensor, Pool tensor_tensor) are
    avoided on purpose: on this hardware they pair-accumulate
    (out[2i] = a[2i]b[2i] + a[2i+1]b[2i+1]) on lanes 84-95/116-127 under
    concurrent load, on every dtype combination tested.
  - exp on ACT with no max-subtraction (|scaled scores| < ~10, safe),
    f32r out; exp(scale*(qk - 32768*m)) underflows to exactly 0 on
    masked lanes.
  - Softmax denominator comes free from the ones column of V':
    outT = V'.T @ attn^T accumulates [H+1, sq] where row H is the row sum.
  - Fixup per q block: 4 batched PE transposes into ONE PSUM bank tile,
    reciprocal + scale on DVE (per-partition-scalar ops only), one
    gathered DMA out.

Phase B runs as one flat pipeline over all 128 (qb, g) groups with attn@V
trailing the scores/exp chain by TRAIL groups, so the PE (the bottleneck
engine) never stalls and stays at the 2.4 GHz pstate.
"""
import sys

sys.path.insert(0, "/opt/trn_rl_repo")

import ml_dtypes
import numpy as np

import concourse.bacc as bacc
import concourse.tile as tile
from concourse import mybir
from concourse.bass_utils import run_bass_kernel_spmd

F32 = mybir.dt.float32
F32R = mybir.dt.float32r
BF16 = mybir.dt.bfloat16

B, S, E, H = 8, 4096, 512, 64
SCALE = float(E) ** -0.5
NEG = -32768.0

BF16NP = ml_dtypes.bfloat16

TRAIL = 2   # attn@V trails scores/exp by this many [128,1024] groups


def build_program(s=S):
    nc = bacc.Bacc("TRN2", target_bir_lowering=False, debug=False, num_devices=B)
    xT = nc.dram_tensor("xT", [E, s], F32R, kind="ExternalInput")
    maskb = nc.dram_tensor("maskb", [s, s], BF16, kind="ExternalInput")
    wq = nc.dram_tensor("wq", [E, H], F32R, kind="ExternalInput")
    wk = nc.dram_tensor("wk", [E, H], F32R, kind="ExternalInput")
    wv = nc.dram_tensor("wv", [E, H], F32R, kind="ExternalInput")
    bqt = nc.dram_tensor("bqt", [H, 1], F32, kind="ExternalInput")
    bkt = nc.dram_tensor("bkt", [H, 1], F32, kind="ExternalInput")
    bv1 = nc.dram_tensor("bv1", [1, H + 1], F32, kind="ExternalInput")
    out = nc.dram_tensor("out", [s, H], F32, kind="ExternalOutput")

    NE = E // 128          # 4 E-chunks
    NB = s // 512          # q/s blocks of 512
    NQ = s // 128          # 128-row chunks
    NG = NQ // 2           # [128,1024]-score groups per q block
    GQ = NB * NG           # total groups

    with tile.TileContext(nc) as tc:
        with (
            tc.tile_pool(name="const", bufs=1) as cst,
            tc.tile_pool(name="xp", bufs=2) as xp,
            tc.tile_pool(name="qkv", bufs=1) as qkv,
            tc.tile_pool(name="maskp", bufs=7) as maskp,
            tc.tile_pool(name="etp", bufs=3) as etp,
            tc.tile_pool(name="osb", bufs=2) as osb,
        ):
            # ---- constants ----
            negI = cst.tile([128, 128], BF16)
            nc.gpsimd.memset(negI, 0.0)
            nc.gpsimd.affine_select(
                out=negI, in_=negI, compare_op=mybir.AluOpType.not_equal,
                fill=NEG, base=0, pattern=[[-1, 128]], channel_multiplier=1,
            )
            idf = cst.tile([128, 128], F32)
            nc.gpsimd.memset(idf, 0.0)
            nc.gpsimd.affine_select(
                out=idf, in_=idf, compare_op=mybir.AluOpType.not_equal,
                fill=1.0, base=0, pattern=[[-1, 128]], channel_multiplier=1,
            )
            ones128 = cst.tile([1, 128], F32)
            nc.vector.memset(ones128, 1.0)

            wq_r = cst.tile([128, NE, H], F32R)
            wk_r = cst.tile([128, NE, H], F32R)
            wv_r = cst.tile([128, NE, H], F32R)
            for w_dram, w_r in ((wq, wq_r), (wk, wk_r), (wv, wv_r)):
                nc.sync.dma_start(
                    out=w_r, in_=w_dram.rearrange("(c p) h -> p c h", p=128)
                )
            bv1_sb = cst.tile([1, H + 1], F32)
            nc.sync.dma_start(out=bv1_sb, in_=bv1[:])
            bqt_sb = cst.tile([H, 1], F32)
            bkt_sb = cst.tile([H, 1], F32)
            nc.sync.dma_start(out=bqt_sb, in_=bqt[:])
            nc.sync.dma_start(out=bkt_sb, in_=bkt[:])

            # ---- mask DMA (issue first: no deps, fills DMA queues early) ----
            # mbs[qb][j]: mask rows for sq chunk (qb, j) as bf16 {0,1},
            # [128, s] — plain 2D row-slice DMAs.
            mbs = []
            for qb in range(NB):
                row = []
                for j in range(4):
                    q0 = qb * 512 + j * 128
                    mb = maskp.tile([128, s], BF16, tag="mb", name=f"mb_{qb}_{j}")
                    nc.sync.dma_start(out=mb, in_=maskb[q0:q0 + 128, :])
                    row.append(mb)
                mbs.append(row)

            # ---- phase A: QT, KT head-major; V' S-major (all f32r) ----
            QTb = [qkv.tile([H, 512], F32R, name=f"qt_{i}") for i in range(NB)]
            KTb = [qkv.tile([H, 512], F32R, name=f"kt_{i}") for i in range(NB)]
            VPk = [qkv.tile([128, H + 1], F32R, name=f"vp_{i}") for i in range(NQ)]
            with tc.tile_pool(name="psA", bufs=2, space="PSUM") as psA:
                for sb in range(NB):
                    s0 = sb * 512
                    xtr = xp.tile([128, NE, 512], F32R, tag="xtr", name=f"xtr_{sb}")
                    half = NE // 2
                    for eh in range(2):
                        e0 = eh * half
                        eng = nc.scalar if eh == 0 else nc.gpsimd
                        eng.dma_start(
                            out=xtr[:, e0:e0 + half, :],
                            in_=xT[e0 * 128:(e0 + half) * 128, s0:s0 + 512]
                            .rearrange("(c p) s -> p c s", p=128),
                        )
                    q_ps = psA.tile([H, 512], F32, tag="qk", name=f"q_ps_{sb}")
                    k_ps = psA.tile([H, 512], F32, tag="qk", name=f"k_ps_{sb}")
                    for e in range(NE):
                        nc.tensor.matmul(q_ps, wq_r[:, e, :], xtr[:, e, :],
                                         start=(e == 0), stop=(e == NE - 1))
                        nc.tensor.matmul(k_ps, wk_r[:, e, :], xtr[:, e, :],
                                         start=(e == 0), stop=(e == NE - 1))
                    nc.scalar.activation(QTb[sb], q_ps,
                                         mybir.ActivationFunctionType.Identity,
                                         bias=bqt_sb)
                    nc.scalar.activation(KTb[sb], k_ps,
                                         mybir.ActivationFunctionType.Identity,
                                         bias=bkt_sb)
                    for j0 in range(0, 4, 2):
                        vps = [
                            psA.tile([128, H + 1], F32, tag="v",
                                     name=f"v_ps_{sb}_{j0 + jj}")
                            for jj in range(2)
                        ]
                        for jj in range(2):
                            nc.tensor.matmul(vps[jj], ones128, bv1_sb,
                                             start=True, stop=False)
                        for e in range(NE):
                            for jj in range(2):
                                c0 = (j0 + jj) * 128
                                nc.tensor.matmul(
                                    vps[jj][:, 0:H], xtr[:, e, c0:c0 + 128],
                                    wv_r[:, e, :], start=False, stop=(e == NE - 1),
                                )
                        for jj in range(2):
                            nc.vector.tensor_copy(VPk[sb * 4 + j0 + jj], vps[jj])

            # ---- phase B: flat pipeline over all (qb, g) groups ----
            with (
                tc.tile_pool(name="psS", bufs=3, space="PSUM") as psS,
                tc.tile_pool(name="psO", bufs=1, space="PSUM") as psO,
            ):
                ot_ps = [None] * NB

                def scores(G):
                    qb, g = divmod(G, NG)
                    sc = psS.tile([128, 1024], F32, tag="sc", name=f"sc_{G}")
                    for h2 in range(2):
                        k = 2 * g + h2
                        nc.tensor.matmul(
                            sc[:, 512 * h2:512 * h2 + 512],
                            KTb[k // 4][:, 128 * (k % 4):128 * (k % 4 + 1)],
                            QTb[qb],
                            start=True, stop=False,
                        )
                    for j in range(4):
                        for h2 in range(2):
                            k = 2 * g + h2
                            c = 512 * h2 + 128 * j
                            nc.tensor.matmul(
                                sc[:, c:c + 128],
                                mbs[qb][j][:, 128 * k:128 * (k + 1)],
                                negI, start=False, stop=(j == 3),
                            )
                    return sc

                def expg(G, sc):
                    et = etp.tile([128, 1024], F32R, tag="et")
                    nc.scalar.activation(
                        et, sc, mybir.ActivationFunctionType.Exp, scale=SCALE
                    )
                    return et

                def attnv(G, et):
                    qb, g = divmod(G, NG)
                    if ot_ps[qb] is None:
                        ot_ps[qb] = psO.tile([H + 1, 512], F32, tag="ot",
                                             name=f"ot_{qb}")
                    for h2 in range(2):
                        k = 2 * g + h2
                        nc.tensor.matmul(
                            ot_ps[qb], VPk[k], et[:, 512 * h2:512 * h2 + 512],
                            start=(k == 0), stop=(k == NQ - 1),
                        )

                def fixup(qb):
                    q0 = qb * 512
                    oT = osb.tile([H + 1, 512], F32, tag="oT")
                    nc.vector.tensor_copy(oT, ot_ps[qb])
                    fx = psS.tile([128, 4, H + 1], F32, tag="fx", bufs=1)
                    for j in range(4):
                        nc.tensor.transpose(
                            fx[:, j, :], oT[:, 128 * j:128 * (j + 1)],
                            idf[0:H + 1, 0:H + 1]
                        )
                    ob = osb.tile([128, 4, H + 1], F32, tag="ob")
                    nc.vector.tensor_copy(ob, fx)
                    rc = osb.tile([128, 4], F32, tag="rc")
                    nc.vector.reciprocal(rc, ob[:, :, H])
                    of = osb.tile([128, 4, H], F32, tag="of")
                    for j in range(4):
                        nc.vector.tensor_scalar_mul(
                            of[:, j, :], ob[:, j, 0:H], rc[:, j:j + 1]
                        )
                    nc.gpsimd.dma_start(
                        out=out[q0:q0 + 512, :].rearrange("(j p) h -> p j h", p=128),
                        in_=of,
                    )

                ets = {}
                scn = {0: scores(0)}
                for G in range(GQ):
                    if G + 1 < GQ:
                        scn[G + 1] = scores(G + 1)
                    ets[G] = expg(G, scn.pop(G))
                    if G - TRAIL >= 0:
                        attnv(G - TRAIL, ets.pop(G - TRAIL))
                        if (G - TRAIL) % NG == NG - 1:
                            fixup((G - TRAIL) // NG)
                for G in range(GQ - TRAIL, GQ):
                    attnv(G, ets.pop(G))
                    if G % NG == NG - 1:
                        fixup(G // NG)
    nc.compile()
    return nc


def make_in_maps(x, attention_mask, Wq, bq, Wk, bk, Wv, bv):
    nb = x.shape[0]
    bv1 = np.concatenate([bv, np.ones(1, np.float32)]).reshape(1, H + 1)
    common = {
        "wq": np.ascontiguousarray(Wq), "wk": np.ascontiguousarray(Wk),
        "wv": np.ascontiguousarray(Wv),
        "bqt": np.ascontiguousarray(bq.reshape(H, 1)),
        "bkt": np.ascontiguousarray(bk.reshape(H, 1)),
        "bv1": bv1,
    }
    return [
        {
            "xT": np.ascontiguousarray(x[b].T),
            # mask as bf16 {0,1}, exact
            "maskb": (attention_mask[b] != 0).astype(BF16NP),
            **common,
        }
        for b in range(nb)
    ]


_PROGRAM = None


def kernel(x, attention_mask, Wq, bq, Wk, bk, Wv, bv):
    global _PROGRAM
    x = np.asarray(x, np.float32)
    attention_mask = np.asarray(attention_mask, np.int32)
    if _PROGRAM is None:
        _PROGRAM = build_program()
    in_maps = make_in_maps(
        x, attention_mask,
        np.asarray(Wq, np.float32), np.asarray(bq, np.float32),
        np.asarray(Wk, np.float32), np.asarray(bk, np.float32),
        np.asarray(Wv, np.float32), np.asarray(bv, np.float32),
    )
    res = run_bass_kernel_spmd(_PROGRAM, in_maps, core_ids=list(range(B)))
    return np.stack([res.results[b]["out"] for b in range(B)], axis=0)
